# revision 41
# baseline (speedup 1.0000x reference)
"""Trainium2 Bass kernel for nn_BlockRF (BatchNorm -> LocallyConnected2D 3x3 valid -> ReLU).

Shapes (hardcoded per the problem spec):
  x:      [B=32, H=64, W=64, C=32]  f32
  gamma/beta/moving_mean/moving_var: [C=32] f32
  weight: [OH*OW=3844, KH*KW*C=288, F=32] f32
  out:    [B=32, OH=62, OW=62, F=32] f32

Strategy: shard over output rows (OH). OH=62 padded to 64 -> 8 rows/core on 8
cores. Each core streams only its slice of the (dominant) weight tensor, fp8
E3M4 with one global scale sw folded into the BN affine (see below).

Layout: the contraction is K=128 = 4 taps x 32 channels over a 4-H-row x
window; each output row uses 3 real taps and one all-zero weight tap (even
rows zero tap 3, odd rows zero tap 0, baked in on host).  Why K=128 and not
the minimal K=96: (a) compute-engine SBUF access patterns must start at
partition 0 to span >32 partitions, so a 96-row read of the shared window at
offset 32 is illegal; (b) SDMA engine<->partition wiring serves partitions
0-63 on even engines and 64-127 on odd ones, so 96-partition transfers cap
the DMA at ~2/3 of peak while 128-partition transfers are balanced.  The
zero bands cost +1.5 MB/core but land mostly on otherwise-idle engine
parity, leaving the binding even-engine byte load unchanged.  Matmul time
is N*cycle, independent of K, so the zero taps cost no PE time.

DMA plan (the binding resource):
  - x rides gpsimd/SWDGE as 4 overlapping window tiles [(row,c)=128,
    (w,b)=4096B]: window k holds H-rows 2k..2k+3 and serves output rows 2k
    and 2k+1, cutting x bytes 3.15 -> 2.1 MB/core.
  - weights stream per-oh as two half tiles [128, 2976B]; even oh on the
    sync HWDGE ring, odd oh on the scalar HWDGE ring, so both rings
    generate descriptors concurrently.
  - y is evacuated ReLU'd as ONE [128, 512] activation per row (all 4 PSUM
    position groups at once) into a shared [128, 4-row] fp16 tile; two
    balanced [128, 4096B] stores ride the gpsimd ring after its x loads
    finish. Host un-permutes the (group, batch, row) packing.
  - all DMA dispatches are emitted before any compute op, so descriptor
    generation is never head-of-line blocked on an engine.

Matmul structure per output row oh (pipelined via tile pools):
  - For each stationary w-slice (fp16 x cols [128, 32]): 1-2 accumulating
    matmuls, moving = fp8e3 weight chunks, N<=96, fp32 PSUM, tile_position
    (0, 32*grp) - one single-bank [128,512] PSUM tile per row, 4 position
    groups of 16 at partition offsets 0/32/64/96; memset-ed (PSUM
    start=True would pend-zero the whole bank, breaking interleaved
    accumulation).
  - a short warmup burst of zero-matmuls at kernel start keeps the PE HAM
    clock gate (1.2 -> 2.4 GHz) favorably phased while the first DMAs
    stream.

Host side only pads/transposes/casts/quantizes (layout prep + sharding) - all
model arithmetic (BN, conv, ReLU) runs on device.
"""

import numpy as np

B, H, W, C, F = 32, 64, 64, 32, 32
KH = KW = 3
OH = OW = 62
OHP = 64  # padded OH
RPC = OHP // 8  # output rows per core = 8
NWIN = RPC // 2  # x window tiles per core = 4
EPS = 1e-3
XFREE = W * B  # 2048
FP8_MAX = 15.5  # max finite |value| of E3M4
WARMUP_MM = 4  # N=512 zero-matmuls to lift the HAM clock gate
# packed weight slots: for stationary w-slice, valid g values are those with
# ow = w-2+g in [0, OW); slots stored g-ascending, w-major
_GMIN = [max(0, 2 - w) for w in range(W)]
_GMAX = [min(2, OW - 1 - (w - 2)) for w in range(W)]
_SLOT_BASE = [0] * W
for _w in range(1, W):
    _SLOT_BASE[_w] = _SLOT_BASE[_w - 1] + (_GMAX[_w - 1] - _GMIN[_w - 1] + 1)
NSLOT = _SLOT_BASE[-1] + (_GMAX[-1] - _GMIN[-1] + 1)  # 186
WSLOT = NSLOT * F  # 5952
PSUM_POS = 16  # positions per PSUM partition-group (16*32*4B = 2KB = 1 bank)
YROW = PSUM_POS * F  # 512 fp16 elements per row in the packed y tile

_CACHE = {}


def _build_program():
    import concourse.mybir as mybir
    import concourse.tile as tile
    from concourse import bacc
    from contextlib import ExitStack

    f16 = mybir.dt.float16
    f32 = mybir.dt.float32
    f8 = mybir.dt.float8e3

    nc = bacc.Bacc("TRN2", target_bir_lowering=False, debug=False, num_devices=8)

    xin = nc.dram_tensor("xin", [NWIN, 128, XFREE], f16, kind="ExternalInput").ap()
    win = nc.dram_tensor("win", [RPC, 96, WSLOT], f8, kind="ExternalInput").ap()
    pin = nc.dram_tensor("pin", [128, 5], f32, kind="ExternalInput").ap()
    yout = nc.dram_tensor("yout", [2, 128, 4 * YROW], f16, kind="ExternalOutput").ap()

    HW = WSLOT // 2  # = 93*F: exactly the slots of w<32

    with ExitStack() as ctx:
        tc = ctx.enter_context(tile.TileContext(nc))
        singles = ctx.enter_context(tc.tile_pool(name="singles", bufs=1))
        xpool = ctx.enter_context(tc.tile_pool(name="xpool", bufs=NWIN))
        xnpool = ctx.enter_context(tc.tile_pool(name="xnpool", bufs=NWIN))
        wpool = ctx.enter_context(tc.tile_pool(name="wpool", bufs=16))
        opool = ctx.enter_context(tc.tile_pool(name="opool", bufs=2))
        pspool = ctx.enter_context(
            tc.tile_pool(name="pspool", bufs=6, space="PSUM")
        )

        # ---- all DMA dispatches first, so no compute op ever head-of-line
        # blocks a descriptor-generation trigger on its engine.
        # BN affine params (scaled by the fp8 dequant scale sw, pin col 4)
        # ride the scalar ring ahead of the odd-row weights (2.5 KB).
        par = singles.tile([128, 5], f32)
        nc.scalar.dma_start(out=par, in_=pin)

        # weights: even oh -> sync ring, odd oh -> scalar ring. Two
        # sequential half tiles per row keep the per-row matmul burst able
        # to start half a transfer early.  Only the 96 real partitions are
        # shipped (-1.5 MB/core of zeros): even rows land at partitions
        # 0-95 and contract with K=96; odd rows land at 32-127, their
        # 0-31 zero band is memset on-chip, and they contract with K=128.
        # The concurrent opposite-parity bands also balance the even/odd
        # SDMA engine loads.
        wts = []
        for oh in range(RPC):
            eng = nc.sync if oh % 2 == 0 else nc.scalar
            b0 = 32 * (oh % 2)
            wta = wpool.tile([128, HW], f8, name="wta", tag="wt")
            wtb = wpool.tile([128, WSLOT - HW], f8, name="wtb", tag="wt")
            eng.dma_start(out=wta[b0:b0 + 96, :], in_=win[oh][:, :HW])
            eng.dma_start(out=wtb[b0:b0 + 96, :], in_=win[oh][:, HW:])
            wts.append((wta, wtb))

        # x windows on the gpsimd SWDGE ring (no y stores ahead of them)
        xts = []
        for k in range(NWIN):
            xt = xpool.tile([128, XFREE], f16, name="xt", tag="xt")
            nc.gpsimd.dma_start(out=xt, in_=xin[k])
            xts.append(xt)

        # PE warmup: zero-matmuls keep the PE array visibly busy while the
        # first DMAs stream, so the HAM clock gate opens (1.2 -> 2.4 GHz)
        # before the real matmuls start
        zt = singles.tile([96, 512], f16)
        nc.vector.memset(zt, 0.0)
        wups = pspool.tile([128, 512], mybir.dt.float32, name="wup", tag="ps")
        for _ in range(WARMUP_MM):
            nc.tensor.matmul(
                wups[:B], zt[:, :B], zt, start=True, stop=True,
                skip_group_check=True,
            )

        # A = sw*gamma/sqrt(var+eps), Bb = sw*(beta - mean*A0)
        tmp = singles.tile([128, 1], f32)
        A = singles.tile([128, 1], f32)
        Bb = singles.tile([128, 1], f32)
        nc.vector.tensor_scalar_add(tmp, par[:, 3:4], EPS)  # var + eps
        nc.scalar.sqrt(tmp, tmp)
        nc.vector.reciprocal(A, tmp)  # 1/sqrt(var+eps)
        nc.vector.tensor_mul(A, A, par[:, 0:1])  # * gamma
        nc.vector.tensor_mul(tmp, A, par[:, 2:3])  # mean * A
        nc.vector.tensor_sub(Bb, par[:, 1:2], tmp)  # beta - mean*A
        nc.vector.tensor_mul(A, A, par[:, 4:5])  # * sw
        nc.vector.tensor_mul(Bb, Bb, par[:, 4:5])  # * sw

        # one single-bank PSUM tile per row; first 6 memsets go out up front
        # (pool has 6 bufs), rows 6/7 re-memset recycled banks in the loop
        psts = []
        for oh in range(RPC):
            pst = pspool.tile([128, PSUM_POS * F], mybir.dt.float32,
                              name="ps", tag="ps")
            psts.append(pst)
            if oh < 6:
                nc.vector.memset(pst, 0.0)

        xns = [None] * NWIN
        yts = [opool.tile([128, 4 * YROW], f16, name=f"yt{s}") for s in range(2)]

        for oh in range(RPC):
            wta, wtb = wts[oh]
            kdim = 96 + 32 * (oh % 2)
            pst = psts[oh]
            if oh >= 6:
                nc.vector.memset(pst, 0.0)
            if oh % 2 == 0:
                # zero the NEXT (odd) row's weight band first: dep-free, it
                # fills vector idle time while this row's BN waits on its x
                # window, and lands well before the odd row's matmuls
                nwta, nwtb = wts[oh + 1]
                nc.vector.memset(nwta[:32, :], 0.0)
                nc.vector.memset(nwtb[:32, :], 0.0)
                k = oh // 2
                xn = xnpool.tile([128, XFREE], f16)
                nc.vector.tensor_scalar(
                    xn, xts[k], A, Bb,
                    op0=mybir.AluOpType.mult, op1=mybir.AluOpType.add,
                )
                xns[k] = xn
            xn = xns[oh // 2]

            def emit(w, ow_lo, ow_hi):
                # one matmul covering positions ow_lo..ow_hi (inclusive) at
                # stationary w-slice; slots (w, g=2-(w-ow)) are
                # free-contiguous for ascending ow
                grp = ow_lo // PSUM_POS
                s = ow_lo - grp * PSUM_POS
                n = ow_hi - ow_lo + 1
                g_lo = 2 - (w - ow_lo)
                slot = _SLOT_BASE[w] + (g_lo - _GMIN[w])
                if slot * F < HW:
                    wsrc = wta[:kdim, slot * F:(slot + n) * F]
                else:
                    wsrc = wtb[:kdim, slot * F - HW:(slot + n) * F - HW]
                nc.tensor.matmul(
                    pst[32 * grp:32 * grp + B, s * F:(s + n) * F],
                    xn[:kdim, w * B:(w + 1) * B],
                    wsrc,
                    start=False,
                    stop=True,
                    skip_group_check=True,
                    tile_position=(0, 32 * grp),
                )

            for w in range(W):
                lo, hi = max(w - 2, 0), min(w, OW - 1)
                if lo > hi:
                    continue
                mid = (lo // PSUM_POS) * PSUM_POS + PSUM_POS - 1
                if hi <= mid:
                    emit(w, lo, hi)
                else:  # straddles a PSUM bank/group line
                    emit(w, lo, mid)
                    emit(w, mid + 1, hi)

            # ReLU evacuation: all 4 position groups in one [128,512] pass
            yt = yts[oh // 4]
            nc.scalar.activation(
                yt[:, (oh % 4) * YROW:(oh % 4 + 1) * YROW], pst,
                mybir.ActivationFunctionType.Relu,
            )
            if oh % 4 == 3:
                nc.gpsimd.dma_start(out=yout[oh // 4], in_=yt)

    nc.compile()
    return nc


def _get_program():
    if "nc" not in _CACHE:
        _CACHE["nc"] = _build_program()
    return _CACHE["nc"]


def _prep_inputs(x, gamma, beta, moving_mean, moving_var, weight):
    """Host-side shard/layout/cast/quantize prep. Returns per-core in_maps."""
    import ml_dtypes

    x = np.asarray(x, dtype=np.float32)
    weight = np.asarray(weight, dtype=np.float32)

    # x: [B,H,W,C] -> pad H to 66 -> transpose to (h, c, w, b), fp16
    xpad = np.zeros((B, H + 2, W, C), np.float32)
    xpad[:, :H] = x
    xt_all = np.ascontiguousarray(xpad.transpose(1, 3, 2, 0)).astype(np.float16)
    xt_all = xt_all.reshape(H + 2, C, XFREE)

    # global symmetric E3M4 scale for the weight
    sw = np.float32(np.abs(weight).max() / FP8_MAX)
    if sw == 0.0:
        sw = np.float32(1.0)
    wq = (weight / sw).astype(ml_dtypes.float8_e3m4)
    wq = wq.reshape(OH, OW, KH, KW, C, F)

    # -> (oh, i, c, ow, j, f) then packed slots (w, g): ow=w-2+g, tap j=2-g
    wtr = np.ascontiguousarray(wq.transpose(0, 2, 4, 1, 3, 5))
    wg = np.zeros((OHP, KH, C, NSLOT, F), ml_dtypes.float8_e3m4)
    for w in range(W):
        for g in range(_GMIN[w], _GMAX[w] + 1):
            j = 2 - g
            ow = w - 2 + g
            slot = _SLOT_BASE[w] + (g - _GMIN[w])
            wg[:OH, :, :, slot, :] = wtr[:, :, :, ow, j, :]

    p128 = np.tile(
        np.stack([gamma, beta, moving_mean, moving_var,
                  np.full_like(gamma, sw)], axis=1).astype(np.float32),
        (4, 1),
    )  # [128, 5]

    in_maps = []
    for k in range(8):
        R = k * RPC
        xc = np.stack(
            [xt_all[R + 2 * kw: R + 2 * kw + 4].reshape(128, XFREE)
             for kw in range(NWIN)]
        )  # [NWIN, 128, 2048]
        wc = np.ascontiguousarray(wg[R: R + RPC]).reshape(RPC, 96, WSLOT)
        in_maps.append({"xin": xc, "win": wc, "pin": p128})
    return in_maps


def _assemble_output(results):
    """results: per core {"yout": [2, 128, 2048] f16} -> [B,OH,OW,F] f32.

    yout element (s, 32*g + b, 512*r + 32*owl + f) holds
    y[b, R + 4*s + r, 16*g + owl, f]."""
    nc_ = len(results)
    yall = np.stack([np.asarray(r["yout"]) for r in results])  # [nc,2,128,2048]
    y = yall.astype(np.float32).reshape(nc_, 2, 4, B, 4, PSUM_POS, F)
    # [core, s, g, b, r, owl, f] -> [b, core, s, r, g, owl, f]
    y = y.transpose(3, 0, 1, 4, 2, 5, 6).reshape(B, nc_ * RPC, 4 * PSUM_POS, F)
    return np.ascontiguousarray(y[:, :OH, :OW])


def run(inputs, trace=False, trace_cores=None):
    """Build/compile/run on 8 cores. Returns (y, BassKernelResults)."""
    from concourse.bass_utils import run_bass_kernel_spmd

    nc = _get_program()
    in_maps = _prep_inputs(**inputs)
    res = run_bass_kernel_spmd(
        nc,
        in_maps,
        core_ids=list(range(8)),
        trace=trace,
        **({"trace_cores": trace_cores} if trace_cores is not None else {}),
    )
    return _assemble_output(res.results), res


def kernel(x, gamma, beta, moving_mean, moving_var, weight):
    y, _ = run(
        dict(x=x, gamma=gamma, beta=beta, moving_mean=moving_mean,
             moving_var=moving_var, weight=weight)
    )
    return y


# revision 42
# speedup vs baseline: 1.1678x; 1.1678x over previous
"""Trainium2 Bass kernel for nn_BlockRF (BatchNorm -> LocallyConnected2D 3x3 valid -> ReLU).

Shapes (hardcoded per the problem spec):
  x:      [B=32, H=64, W=64, C=32]  f32
  gamma/beta/moving_mean/moving_var: [C=32] f32
  weight: [OH*OW=3844, KH*KW*C=288, F=32] f32
  out:    [B=32, OH=62, OW=62, F=32] f32

Strategy: shard over output rows (OH). OH=62 padded to 64 -> 8 rows/core on 8
cores. Each core streams only its slice of the (dominant) weight tensor, fp8
E3M4 with one global scale sw folded into the BN affine (see below).

Layout: the contraction is K=128 = 4 taps x 32 channels over a 4-H-row x
window; each output row uses 3 real taps and one all-zero weight tap (even
rows zero tap 3, odd rows zero tap 0, baked in on host).  Why K=128 and not
the minimal K=96: (a) compute-engine SBUF access patterns must start at
partition 0 to span >32 partitions, so a 96-row read of the shared window at
offset 32 is illegal; (b) SDMA engine<->partition wiring serves partitions
0-63 on even engines and 64-127 on odd ones, so 96-partition transfers cap
the DMA at ~2/3 of peak while 128-partition transfers are balanced.  The
zero bands cost +1.5 MB/core but land mostly on otherwise-idle engine
parity, leaving the binding even-engine byte load unchanged.  Matmul time
is N*cycle, independent of K, so the zero taps cost no PE time.

DMA plan (the binding resource):
  - x rides gpsimd/SWDGE as 4 overlapping window tiles [(row,c)=128,
    (w,b)=4096B]: window k holds H-rows 2k..2k+3 and serves output rows 2k
    and 2k+1, cutting x bytes 3.15 -> 2.1 MB/core.
  - weights stream per-oh as two half tiles [128, 2976B]; even oh on the
    sync HWDGE ring, odd oh on the scalar HWDGE ring, so both rings
    generate descriptors concurrently.
  - y is evacuated ReLU'd as ONE [128, 512] activation per row (all 4 PSUM
    position groups at once) into a shared [128, 4-row] fp16 tile; two
    balanced [128, 4096B] stores ride the gpsimd ring after its x loads
    finish. Host un-permutes the (group, batch, row) packing.
  - all DMA dispatches are emitted before any compute op, so descriptor
    generation is never head-of-line blocked on an engine.

Matmul structure per output row oh (pipelined via tile pools):
  - For each stationary w-slice (fp16 x cols [128, 32]): 1-2 accumulating
    matmuls, moving = fp8e3 weight chunks, N<=96, fp32 PSUM, tile_position
    (0, 32*grp) - one single-bank [128,512] PSUM tile per row, 4 position
    groups of 16 at partition offsets 0/32/64/96; memset-ed (PSUM
    start=True would pend-zero the whole bank, breaking interleaved
    accumulation).
  - a short warmup burst of zero-matmuls at kernel start keeps the PE HAM
    clock gate (1.2 -> 2.4 GHz) favorably phased while the first DMAs
    stream.

Host side only pads/transposes/casts/quantizes (layout prep + sharding) - all
model arithmetic (BN, conv, ReLU) runs on device.
"""

import numpy as np

B, H, W, C, F = 32, 64, 64, 32, 32
KH = KW = 3
OH = OW = 62
OHP = 64  # padded OH
RPC = OHP // 8  # output rows per core = 8
NWIN = RPC // 2  # x window tiles per core = 4
EPS = 1e-3
XFREE = W * B  # 2048
FP8_MAX = 15.5  # max finite |value| of E3M4
WARMUP_MM = 4  # N=512 zero-matmuls to lift the HAM clock gate
# packed weight slots: for stationary w-slice, valid g values are those with
# ow = w-2+g in [0, OW); slots stored g-ascending, w-major
_GMIN = [max(0, 2 - w) for w in range(W)]
_GMAX = [min(2, OW - 1 - (w - 2)) for w in range(W)]
_SLOT_BASE = [0] * W
for _w in range(1, W):
    _SLOT_BASE[_w] = _SLOT_BASE[_w - 1] + (_GMAX[_w - 1] - _GMIN[_w - 1] + 1)
NSLOT = _SLOT_BASE[-1] + (_GMAX[-1] - _GMIN[-1] + 1)  # 186
WSLOT = NSLOT * F  # 5952
PSUM_POS = 16  # positions per PSUM partition-group (16*32*4B = 2KB = 1 bank)
YROW = PSUM_POS * F  # 512 fp16 elements per row in the packed y tile

_CACHE = {}


def _build_program():
    import concourse.mybir as mybir
    import concourse.tile as tile
    from concourse import bacc
    from contextlib import ExitStack

    f16 = mybir.dt.float16
    f32 = mybir.dt.float32
    f8 = mybir.dt.float8e3

    nc = bacc.Bacc("TRN2", target_bir_lowering=False, debug=False, num_devices=8)

    xin = nc.dram_tensor("xin", [NWIN, 128, XFREE], f16, kind="ExternalInput").ap()
    win = nc.dram_tensor("win", [RPC, 128, WSLOT], f8, kind="ExternalInput").ap()
    pin = nc.dram_tensor("pin", [128, 5], f32, kind="ExternalInput").ap()
    yout = nc.dram_tensor("yout", [2, 128, 4 * YROW], f16, kind="ExternalOutput").ap()

    HW = WSLOT // 2  # = 93*F: exactly the slots of w<32

    with ExitStack() as ctx:
        tc = ctx.enter_context(tile.TileContext(nc))
        singles = ctx.enter_context(tc.tile_pool(name="singles", bufs=1))
        xpool = ctx.enter_context(tc.tile_pool(name="xpool", bufs=NWIN))
        xnpool = ctx.enter_context(tc.tile_pool(name="xnpool", bufs=NWIN))
        wpool = ctx.enter_context(tc.tile_pool(name="wpool", bufs=16))
        opool = ctx.enter_context(tc.tile_pool(name="opool", bufs=2))
        pspool = ctx.enter_context(
            tc.tile_pool(name="pspool", bufs=6, space="PSUM")
        )

        # ---- all DMA dispatches first, so no compute op ever head-of-line
        # blocks a descriptor-generation trigger on its engine.
        # BN affine params (scaled by the fp8 dequant scale sw, pin col 4)
        # ride the scalar ring ahead of the odd-row weights (2.5 KB).
        par = singles.tile([128, 5], f32)
        nc.scalar.dma_start(out=par, in_=pin)

        # weights: even oh -> sync ring, odd oh -> scalar ring. Two
        # sequential half tiles per row keep the per-row matmul burst able
        # to start half a transfer early.
        wts = []
        for oh in range(RPC):
            eng = nc.sync if oh % 2 == 0 else nc.scalar
            wta = wpool.tile([128, HW], f8, name="wta", tag="wt")
            wtb = wpool.tile([128, WSLOT - HW], f8, name="wtb", tag="wt")
            eng.dma_start(out=wta, in_=win[oh][:, :HW])
            eng.dma_start(out=wtb, in_=win[oh][:, HW:])
            wts.append((wta, wtb))

        # x windows on the gpsimd SWDGE ring (no y stores ahead of them)
        xts = []
        for k in range(NWIN):
            xt = xpool.tile([128, XFREE], f16, name="xt", tag="xt")
            nc.gpsimd.dma_start(out=xt, in_=xin[k])
            xts.append(xt)

        # PE warmup: zero-matmuls keep the PE array visibly busy while the
        # first DMAs stream, so the HAM clock gate opens (1.2 -> 2.4 GHz)
        # before the real matmuls start
        zt = singles.tile([96, 512], f16)
        nc.vector.memset(zt, 0.0)
        wups = pspool.tile([128, 512], mybir.dt.float32, name="wup", tag="ps")
        for _ in range(WARMUP_MM):
            nc.tensor.matmul(
                wups[:B], zt[:, :B], zt, start=True, stop=True,
                skip_group_check=True,
            )

        # A = sw*gamma/sqrt(var+eps), Bb = sw*(beta - mean*A0)
        tmp = singles.tile([128, 1], f32)
        A = singles.tile([128, 1], f32)
        Bb = singles.tile([128, 1], f32)
        nc.vector.tensor_scalar_add(tmp, par[:, 3:4], EPS)  # var + eps
        nc.scalar.sqrt(tmp, tmp)
        nc.vector.reciprocal(A, tmp)  # 1/sqrt(var+eps)
        nc.vector.tensor_mul(A, A, par[:, 0:1])  # * gamma
        nc.vector.tensor_mul(tmp, A, par[:, 2:3])  # mean * A
        nc.vector.tensor_sub(Bb, par[:, 1:2], tmp)  # beta - mean*A
        nc.vector.tensor_mul(A, A, par[:, 4:5])  # * sw
        nc.vector.tensor_mul(Bb, Bb, par[:, 4:5])  # * sw

        # one single-bank PSUM tile per row; first 6 memsets go out up front
        # (pool has 6 bufs), rows 6/7 re-memset recycled banks in the loop
        psts = []
        for oh in range(RPC):
            pst = pspool.tile([128, PSUM_POS * F], mybir.dt.float32,
                              name="ps", tag="ps")
            psts.append(pst)
            if oh < 6:
                nc.vector.memset(pst, 0.0)

        xns = [None] * NWIN
        yts = [opool.tile([128, 4 * YROW], f16, name=f"yt{s}") for s in range(2)]

        for oh in range(RPC):
            wta, wtb = wts[oh]
            pst = psts[oh]
            if oh >= 6:
                nc.vector.memset(pst, 0.0)
            if oh % 2 == 0:
                k = oh // 2
                xn = xnpool.tile([128, XFREE], f16)
                nc.vector.tensor_scalar(
                    xn, xts[k], A, Bb,
                    op0=mybir.AluOpType.mult, op1=mybir.AluOpType.add,
                )
                xns[k] = xn
            xn = xns[oh // 2]

            def emit(w, ow_lo, ow_hi):
                # one matmul covering positions ow_lo..ow_hi (inclusive) at
                # stationary w-slice; slots (w, g=2-(w-ow)) are
                # free-contiguous for ascending ow
                grp = ow_lo // PSUM_POS
                s = ow_lo - grp * PSUM_POS
                n = ow_hi - ow_lo + 1
                g_lo = 2 - (w - ow_lo)
                slot = _SLOT_BASE[w] + (g_lo - _GMIN[w])
                if slot * F < HW:
                    wsrc = wta[:, slot * F:(slot + n) * F]
                else:
                    wsrc = wtb[:, slot * F - HW:(slot + n) * F - HW]
                nc.tensor.matmul(
                    pst[32 * grp:32 * grp + B, s * F:(s + n) * F],
                    xn[:, w * B:(w + 1) * B],
                    wsrc,
                    start=False,
                    stop=True,
                    skip_group_check=True,
                    tile_position=(0, 32 * grp),
                )

            for w in range(W):
                lo, hi = max(w - 2, 0), min(w, OW - 1)
                if lo > hi:
                    continue
                mid = (lo // PSUM_POS) * PSUM_POS + PSUM_POS - 1
                if hi <= mid:
                    emit(w, lo, hi)
                else:  # straddles a PSUM bank/group line
                    emit(w, lo, mid)
                    emit(w, mid + 1, hi)

            # ReLU evacuation: all 4 position groups in one [128,512] pass
            yt = yts[oh // 4]
            nc.scalar.activation(
                yt[:, (oh % 4) * YROW:(oh % 4 + 1) * YROW], pst,
                mybir.ActivationFunctionType.Relu,
            )
            if oh % 4 == 3:
                nc.gpsimd.dma_start(out=yout[oh // 4], in_=yt)

    nc.compile()
    return nc


def _get_program():
    if "nc" not in _CACHE:
        _CACHE["nc"] = _build_program()
    return _CACHE["nc"]


def _prep_inputs(x, gamma, beta, moving_mean, moving_var, weight):
    """Host-side shard/layout/cast/quantize prep. Returns per-core in_maps."""
    import ml_dtypes

    x = np.asarray(x, dtype=np.float32)
    weight = np.asarray(weight, dtype=np.float32)

    # x: [B,H,W,C] -> pad H to 66 -> transpose to (h, c, w, b), fp16
    xpad = np.zeros((B, H + 2, W, C), np.float32)
    xpad[:, :H] = x
    xt_all = np.ascontiguousarray(xpad.transpose(1, 3, 2, 0)).astype(np.float16)
    xt_all = xt_all.reshape(H + 2, C, XFREE)

    # global symmetric E3M4 scale for the weight
    sw = np.float32(np.abs(weight).max() / FP8_MAX)
    if sw == 0.0:
        sw = np.float32(1.0)
    wq = (weight / sw).astype(ml_dtypes.float8_e3m4)
    wq = wq.reshape(OH, OW, KH, KW, C, F)

    # -> (oh, i, c, ow, j, f) then packed slots (w, g): ow=w-2+g, tap j=2-g
    wtr = np.ascontiguousarray(wq.transpose(0, 2, 4, 1, 3, 5))
    wg = np.zeros((OHP, KH, C, NSLOT, F), ml_dtypes.float8_e3m4)
    for w in range(W):
        for g in range(_GMIN[w], _GMAX[w] + 1):
            j = 2 - g
            ow = w - 2 + g
            slot = _SLOT_BASE[w] + (g - _GMIN[w])
            wg[:OH, :, :, slot, :] = wtr[:, :, :, ow, j, :]

    # K=128 band layout: even oh -> taps at bands 0..2 (band 3 zero), odd
    # oh -> taps at bands 1..3 (band 0 zero); band i' = x-window H-row index
    wg128 = np.zeros((OHP, 4, C, NSLOT, F), ml_dtypes.float8_e3m4)
    wg128[0::2, 0:KH] = wg[0::2]
    wg128[1::2, 1:KH + 1] = wg[1::2]

    p128 = np.tile(
        np.stack([gamma, beta, moving_mean, moving_var,
                  np.full_like(gamma, sw)], axis=1).astype(np.float32),
        (4, 1),
    )  # [128, 5]

    in_maps = []
    for k in range(8):
        R = k * RPC
        xc = np.stack(
            [xt_all[R + 2 * kw: R + 2 * kw + 4].reshape(128, XFREE)
             for kw in range(NWIN)]
        )  # [NWIN, 128, 2048]
        wc = np.ascontiguousarray(wg128[R: R + RPC]).reshape(RPC, 128, WSLOT)
        in_maps.append({"xin": xc, "win": wc, "pin": p128})
    return in_maps


def _assemble_output(results):
    """results: per core {"yout": [2, 128, 2048] f16} -> [B,OH,OW,F] f32.

    yout element (s, 32*g + b, 512*r + 32*owl + f) holds
    y[b, R + 4*s + r, 16*g + owl, f]."""
    nc_ = len(results)
    yall = np.stack([np.asarray(r["yout"]) for r in results])  # [nc,2,128,2048]
    y = yall.astype(np.float32).reshape(nc_, 2, 4, B, 4, PSUM_POS, F)
    # [core, s, g, b, r, owl, f] -> [b, core, s, r, g, owl, f]
    y = y.transpose(3, 0, 1, 4, 2, 5, 6).reshape(B, nc_ * RPC, 4 * PSUM_POS, F)
    return np.ascontiguousarray(y[:, :OH, :OW])


def run(inputs, trace=False, trace_cores=None):
    """Build/compile/run on 8 cores. Returns (y, BassKernelResults)."""
    from concourse.bass_utils import run_bass_kernel_spmd

    nc = _get_program()
    in_maps = _prep_inputs(**inputs)
    res = run_bass_kernel_spmd(
        nc,
        in_maps,
        core_ids=list(range(8)),
        trace=trace,
        **({"trace_cores": trace_cores} if trace_cores is not None else {}),
    )
    return _assemble_output(res.results), res


def kernel(x, gamma, beta, moving_mean, moving_var, weight):
    y, _ = run(
        dict(x=x, gamma=gamma, beta=beta, moving_mean=moving_mean,
             moving_var=moving_var, weight=weight)
    )
    return y
